# revision 38
# baseline (speedup 1.0000x reference)
"""Causal self-attention (B=2, T=2048, C=1024, H=16) on 8 trn2 NeuronCores.

Sharding: data-parallel over batch (2 groups of 4 cores) x tensor-parallel over
heads (4 heads / core).  v10: QKV projections as fp8e4 DoubleRow matmuls with
hi/lo splitting (each operand = hi + lo fp8 residual, 3 cross products -> full
bf16-class precision at 0.75x the bf16 PE cost; weights pre-scaled x256/x128
against fp8 subnormals, rescale folded into the exp scale and the V ones
column), bf16 scores/P/V/proj, causal mask via a second PE matmul accumulating
-1e8 into diagonal score blocks, paired-head transpose epilogue with merged
yT copies, PE p-state warmup matmuls over the initial DMA window, prioritized
first-chunk DMAs, 3-buffer score PSUM (B-filler accumulator shares the psW
pool), per-chunk ReduceScatters with post-collective DMAs routed off the busy
queues, and Act-assisted final proj bias-adds.
"""

import sys

for _p in ("/opt/trn_rl_repo",):
    if _p not in sys.path:
        sys.path.append(_p)

import numpy as np
from contextlib import ExitStack

import concourse.bass as bass
import concourse.mybir as mybir
import concourse.tile as tile
from concourse import bass_utils

B, T, C, H = 2, 2048, 1024, 16
D = C // H              # 64
N_CORES = 8
GROUPS = [[0, 1, 2, 3], [4, 5, 6, 7]]
HL = 4                  # heads per core
CL = HL * D             # 256 local channels
KC = C // 128           # 8 contraction chunks of 128
NT = T // 512           # 4 token chunks of 512
TOKC = T // 128         # 16 token chunks of 128
F32 = mybir.dt.float32
F32R = mybir.dt.float32r
BF16 = mybir.dt.bfloat16
FP8 = mybir.dt.float8e4
DRMODE = mybir.MatmulPerfMode.DoubleRow
SW_QK = 256.0           # host scale on w_qk/b_qk (fp8 subnormal avoidance)
SW_V = 128.0            # host scale on w_v/b_v
EXP_SCALE = 0.125 / (SW_QK * SW_QK)   # 2^-19 exactly
MASK_NEG = -1.0e8


def _legalize_waits(nc):
    """This walrus build allows at most ONE sync-wait per instruction. Move
    extra waits onto same-engine NoOps inserted just before the instruction."""
    n_split = 0
    for f in nc.m.functions:
        for bb in f.blocks:
            out = []
            for inst in bb.instructions:
                si = inst.sync_info
                waits = list(si.on_wait) if si is not None and si.on_wait else []
                if len(waits) > 1:
                    for i, w in enumerate(waits[:-1]):
                        out.append(
                            mybir.InstNoOp(
                                name=f"wsplit_{inst.name}_{i}",
                                engine=inst.engine,
                                ins=[],
                                outs=[],
                                sync_info=mybir.SyncInfo(on_wait=[w], on_update=[]),
                            )
                        )
                        n_split += 1
                    si.on_wait = [waits[-1]]
                out.append(inst)
            bb.instructions = out
    return n_split


def _build_bass():
    nc = bass.Bass("TRN2", target_bir_lowering=False, debug=False,
                   num_devices=N_CORES)

    xT = nc.dram_tensor("xT", [C, T], FP8, kind="ExternalInput").ap()
    xTl = nc.dram_tensor("xTl", [C, T], FP8, kind="ExternalInput").ap()
    w_qk = nc.dram_tensor("w_qk", [C, 2 * CL], FP8, kind="ExternalInput").ap()
    w_qkl = nc.dram_tensor("w_qkl", [C, 2 * CL], FP8, kind="ExternalInput").ap()
    b_qk = nc.dram_tensor("b_qk", [2 * CL], F32, kind="ExternalInput").ap()
    w_v = nc.dram_tensor("w_v", [C, CL], FP8, kind="ExternalInput").ap()
    w_vl = nc.dram_tensor("w_vl", [C, CL], FP8, kind="ExternalInput").ap()
    b_v = nc.dram_tensor("b_v", [CL], F32R, kind="ExternalInput").ap()
    w_pr = nc.dram_tensor("w_pr", [CL, C], BF16, kind="ExternalInput").ap()
    b_pr = nc.dram_tensor("b_pr", [C], F32, kind="ExternalInput").ap()
    out_rs = nc.dram_tensor("out_rs", [C // 4, T], BF16, kind="ExternalOutput").ap()

    with tile.TileContext(nc) as tc:
        with ExitStack() as ctx:
            with nc.allow_low_precision(reason="fp8/bf16 attention; tolerance 2e-2"):
                _build_body(ctx, tc, nc, xT, xTl, w_qk, w_qkl, b_qk, w_v, w_vl,
                            b_v, w_pr, b_pr, out_rs)

    _legalize_waits(nc)
    return nc


def _build_body(ctx, tc, nc, xT, xTl, w_qk, w_qkl, b_qk, w_v, w_vl, b_v,
                w_pr, b_pr, out_rs):
    Exp = mybir.ActivationFunctionType.Exp

    persist = ctx.enter_context(tc.tile_pool(name="persist", bufs=1))
    dram = ctx.enter_context(tc.tile_pool(name="dram", bufs=1, space="DRAM"))

    # ---- pools (kept open for the whole kernel; no phase barriers) --------
    psW = ctx.enter_context(tc.tile_pool(name="psW", bufs=2, space="PSUM"))
    psS = ctx.enter_context(tc.tile_pool(name="psS", bufs=3, space="PSUM"))
    psPV = ctx.enter_context(tc.tile_pool(name="psPV", bufs=2, space="PSUM"))
    pP = ctx.enter_context(tc.tile_pool(name="pP", bufs=34))
    pR = ctx.enter_context(tc.tile_pool(name="pR", bufs=2))
    pY = ctx.enter_context(tc.tile_pool(name="pY", bufs=10))
    pO = ctx.enter_context(tc.tile_pool(name="pO", bufs=4))

    # ---- PE p-state warmup: dummy matmuls with no DMA deps ----------------
    dummy = persist.tile([128, 512], BF16, name="dummy")
    nc.gpsimd.memset(dummy[:], 0.5)
    for i in range(8):
        scr = psS.tile([128, 512], F32, name="s_ps")
        nc.tensor.matmul(scr[:], lhsT=dummy[:, 0:128], rhs=dummy[:],
                         start=True, stop=True)

    # ---- weight + activation loads, ordered so phase A starts earliest ----
    # fp8 hi/lo: each operand tile holds its hi fp8 part in k-planes 0..3 and
    # the lo residual in k-planes 4..7.  critical path: x(n=0) hi + w_qk hi
    # gate the first A DoubleRow products (hi x hi).
    w_qk_m = w_qk.rearrange("(g k p) c -> g p k c", g=2, k=4)
    w_qkl_m = w_qkl.rearrange("(g k p) c -> g p k c", g=2, k=4)
    xT_m = xT.rearrange("(g k p) (n c) -> n g p k c", g=2, k=4, n=NT)
    xTl_m = xTl.rearrange("(g k p) (n c) -> n g p k c", g=2, k=4, n=NT)
    xT_t = {}

    def load_x(n):
        for g in range(2):
            t = persist.tile([128, 8 * 512], FP8, name=f"xT_m{g}_{n}")
            k8 = t[:].rearrange("p (k c) -> p k c", k=8)
            eng = nc.scalar if (g == 1 and n == 0) else nc.sync
            eng.dma_start(k8[:, 0:4, :], xT_m[n, g])
            xT_t[g, n] = t
        for g in range(2):
            k8 = xT_t[g, n][:].rearrange("p (k c) -> p k c", k=8)
            eng = nc.scalar if (g == 1 and n == 0) else nc.sync
            eng.dma_start(k8[:, 4:8, :], xTl_m[n, g])

    def x_pair(hl, p, n, o, width):
        """[128, 2, width] fp8 DoubleRow operand: kc pair p, hi (0) / lo (1)."""
        t = xT_t[p // 2, n]
        k8 = t[:].rearrange("p (k c) -> p k c", k=8)
        base = 4 * hl + 2 * (p % 2)
        return k8[:, base:base + 2, o:o + width]

    class _PairSlices:
        def __init__(self, tiles, width):
            self.tiles, self.width = tiles, width

        def __getitem__(self, key):
            hl, p, o = key
            t = self.tiles[p // 2]
            k8 = t[:].rearrange("p (k c) -> p k c", k=8)
            base = 4 * hl + 2 * (p % 2)
            return k8[:, base:base + 2, o:o + self.width]

    # critical order: hi halves of x(n=0) + w_qk first, split across the two
    # DMA queues; lo halves right behind (only the 5th+ matmul needs them)
    w_qk_t = []
    t0 = persist.tile([128, 8 * 2 * CL], FP8, name="w_qk_m0")
    nc.sync.dma_start(t0[:].rearrange("p (k c) -> p k c", k=8)[:, 0:4, :],
                      w_qk_m[0])
    w_qk_t.append(t0)
    t1 = persist.tile([128, 8 * 2 * CL], FP8, name="w_qk_m1")
    load_x(0)   # hi g=0 on sync, hi g=1 on scalar, then lo halves
    nc.scalar.dma_start(t1[:].rearrange("p (k c) -> p k c", k=8)[:, 0:4, :],
                        w_qk_m[1])
    w_qk_t.append(t1)
    nc.sync.dma_start(t0[:].rearrange("p (k c) -> p k c", k=8)[:, 4:8, :],
                      w_qkl_m[0])
    nc.scalar.dma_start(t1[:].rearrange("p (k c) -> p k c", k=8)[:, 4:8, :],
                        w_qkl_m[1])
    w_qk_sb = _PairSlices(w_qk_t, 128)
    b_qk_sb = persist.tile([128, 4], F32, name="b_qk_sb")
    nc.sync.dma_start(b_qk_sb[:], b_qk.rearrange("(m p) -> p m", p=128))
    b_v_row = persist.tile([1, CL], F32R, name="b_v_row")
    nc.sync.dma_start(b_v_row[:], b_v.rearrange("(a c) -> a c", a=1))
    w_v_m = w_v.rearrange("(g k p) c -> g p k c", g=2, k=4)
    w_vl_m = w_vl.rearrange("(g k p) c -> g p k c", g=2, k=4)
    w_v_t = []
    for g in range(2):
        t = persist.tile([128, 8 * CL], FP8, name=f"w_v_m{g}")
        k8 = t[:].rearrange("p (k c) -> p k c", k=8)
        nc.sync.dma_start(k8[:, 0:4, :], w_v_m[g])
        nc.sync.dma_start(k8[:, 4:8, :], w_vl_m[g])
        w_v_t.append(t)
    w_v_sb = _PairSlices(w_v_t, CL)
    load_x(1)
    w_prb = []
    for kc in range(2):
        t = persist.tile([128, C], BF16, name=f"w_prb_{kc}")
        nc.sync.dma_start(t[:], w_pr[kc * 128:(kc + 1) * 128, :])
        w_prb.append(t)
    load_x(2)
    load_x(3)
    b_pr_sb = persist.tile([128, 8], F32, name="b_pr_sb")
    nc.sync.dma_start(b_pr_sb[:], b_pr.rearrange("(m p) -> p m", p=128))
    ones_f32 = persist.tile([1, 128], F32, name="ones_f32")
    nc.gpsimd.memset(ones_f32[:], 1.0)
    ones_row = persist.tile([1, 128], F32R, name="ones_row")
    nc.vector.tensor_copy(ones_row[:], ones_f32[:])

    # additive causal mask for the diagonal 128x128 block:
    # maskneg[k,q] = 0 if q>=k else -1e8 (applied via accumulating matmul)
    maskneg = persist.tile([128, 128], BF16, name="maskneg")
    nc.gpsimd.memset(maskneg[:], 0.0)
    nc.gpsimd.affine_select(
        out=maskneg[:], in_=maskneg[:], compare_op=mybir.AluOpType.is_ge,
        fill=MASK_NEG, base=0, pattern=[[1, 128]], channel_multiplier=-1)

    # bf16 identity for PE transposes
    ident = persist.tile([128, 128], BF16, name="ident")
    nc.gpsimd.memset(ident[:], 1.0)
    nc.gpsimd.affine_select(
        out=ident[:], in_=ident[:], compare_op=mybir.AluOpType.is_ge, fill=0.0,
        base=0, pattern=[[1, 128]], channel_multiplier=-1)
    nc.gpsimd.affine_select(
        out=ident[:], in_=ident[:], compare_op=mybir.AluOpType.is_ge, fill=0.0,
        base=0, pattern=[[-1, 128]], channel_multiplier=1)

    # ---- persistent intermediates ----------------------------------------
    # QK_sb[m]: m=0,1 -> Q channels (heads 0,1 | 2,3), m=2,3 -> K channels
    QK_sb = [persist.tile([128, T], BF16, name=f"QK_{m}") for m in range(4)]
    # V in [tok, ch] bf16 layout, 65 cols/head: col h*65+64 is the ones column
    # denominator carries the same x128 scale as the V data columns)
    V_sb = [persist.tile([128, HL * 65], BF16, name=f"V_{t}") for t in range(TOKC)]
    # y^T in bf16, rows = local channels (kc 0: heads 0,1; kc 1: heads 2,3)
    yT_sb = [persist.tile([128, T], BF16, name=f"yT_{i}") for i in range(2)]

    # denominator column carries the same xSW_V scale as the V data columns
    for t in range(TOKC):
        vt = V_sb[t][:].rearrange("p (h c) -> p h c", h=HL)
        nc.gpsimd.memset(vt[:, :, 64:65], SW_V)

    # broadcast b_v across partitions via a rank-1 matmul (emitted as a
    # filler, tag 2, so its b_v_row DMA wait never stalls the early PE stream)
    b_v_bc = persist.tile([128, CL], F32, name="b_v_bc")

    def bvp_run():
        bvp = psW.tile([128, 512], F32, name="a_ps")
        nc.tensor.matmul(bvp[:, 0:CL], lhsT=ones_row[:], rhs=b_v_row[:],
                         start=True, stop=True)
        nc.vector.tensor_copy(b_v_bc[:], bvp[:, 0:CL])

    bounce_n = [dram.tile([C, 512], BF16, name=f"bounce_{n}") for n in range(NT)]
    rs_out_n = [dram.tile([C // 4, 512], BF16, name=f"rs_out_{n}") for n in range(NT)]

    def q_ap(h):
        return QK_sb[h // 2][(h % 2) * 64:(h % 2) * 64 + 64, :]

    def k_ap(h):
        return QK_sb[2 + h // 2][(h % 2) * 64:(h % 2) * 64 + 64, :]

    # ---- static scheduler: PE is the master stream; exps (Act) must never
    # starve.  Fillers are single matmuls injected whenever the PE virtual
    # clock is ahead of the Act virtual clock.
    PE_ROW = 1.0 / 2.4          # ns per output row at full p-state
    ACT_ROW = 1.0 / 1.2
    ACT_FIX = 185.0
    EXP_LAT = 250.0             # sem hop from score-done to exp start

    state = {"pe": 0.0, "act": 0.0}
    fillers = []                # list of (cost_ns, closure, tag)

    def emit_fillers(margin=800.0):
        if state["act"] == 0.0:
            return
        while fillers and state["pe"] < state["act"] + margin:
            cost, run, _tag = fillers.pop(0)
            run()
            state["pe"] += cost

    def drain_fillers(upto_tag=None):
        while fillers and (upto_tag is None or fillers[0][2] is not None
                           and fillers[0][2] <= upto_tag):
            cost, run = fillers.pop(0)[:2]
            run()
            state["pe"] += cost

    # A(m,n): 12 fp8 DoubleRow matmuls (kc pairs x products hh, hl, lh)
    # accumulating into one psW tile, then DVE bias-add into bf16 QK.
    # m order 0,2 (heads 0/1 Q+K) then 1,3 so attention can start earliest.
    A_PRODS = [(0, 0, p) for p in range(4)] + \
              [(0, 1, p) for p in range(4)] + \
              [(1, 0, p) for p in range(4)]
    NA = len(A_PRODS)

    def add_a_fillers(n):
        holder = {}
        for gi, grp in enumerate(((0, 2), (1, 3))):
            for m in grp:
                def mk(m, i):
                    w_hl, x_hl, p = A_PRODS[i]

                    def run():
                        if i == 0:
                            holder[m] = psW.tile([128, 512], F32, name="a_ps")
                        nc.tensor.matmul(
                            holder[m][:], lhsT=w_qk_sb[w_hl, p, m * 128],
                            rhs=x_pair(x_hl, p, n, 0, 512),
                            start=(i == 0), stop=(i == NA - 1),
                            perf_mode=DRMODE)
                        if i == NA - 1:
                            nc.vector.tensor_scalar_add(
                                QK_sb[m][:, n * 512:(n + 1) * 512], holder.pop(m)[:],
                                b_qk_sb[:, m:m + 1])
                    return run
                for i in range(NA):
                    fillers.append((107.0, mk(m, i), 3 * n + gi))

    # B(t,n): 12 DoubleRow matmuls into psW, then DVE bias-add into V bf16.
    B_PRODS = [(0, 0, p) for p in range(4)] + \
              [(1, 0, p) for p in range(4)] + \
              [(0, 1, p) for p in range(4)]

    def add_b_fillers(n):
        holder = {}
        for t in range(4):
            def mk(t, i):
                x_hl, w_hl, p = B_PRODS[i]

                def run():
                    if i == 0:
                        holder[t] = psW.tile([128, 512], F32, name="a_ps")
                    nc.tensor.matmul(
                        holder[t][:, 0:CL],
                        lhsT=x_pair(x_hl, p, n, t * 128, 128),
                        rhs=w_v_sb[w_hl, p, 0],
                        start=(i == 0), stop=(i == NA - 1),
                        perf_mode=DRMODE)
                    if i == NA - 1:
                        vt = V_sb[n * 4 + t][:].rearrange("p (h c) -> p h c", h=HL)
                        nc.vector.tensor_add(
                            vt[:, :, 0:64],
                            holder.pop(t)[:, 0:CL].rearrange(
                                "p (h c) -> p h c", h=HL),
                            b_v_bc[:].rearrange("p (h c) -> p h c", h=HL))
                return run
            for i in range(NA):
                fillers.append((53.0, mk(t, i), 3 * n + 2))

    # D(m,qi): 2 kc-matmuls + DVE bias-add + DMA; last m also issues the RS.
    # The post-RS output DMA goes on the gpsimd (SWDGE) queue: its wait on the
    # collective must not head-of-line-block the bounce DMA queues.
    def d_closures(qi, alt=False):
        out = []
        for m in range(8):
            def mk(m):
                def run():
                    if alt and m % 2 == 1:
                        ps = psS.tile([128, 512], F32, name="s_ps")
                    else:
                        ps = psW.tile([128, 512], F32, name="a_ps")
                    for kc in range(2):
                        nc.tensor.matmul(
                            ps[:], lhsT=w_prb[kc][:, m * 128:(m + 1) * 128],
                            rhs=yT_sb[kc][:, qi * 512:(qi + 1) * 512],
                            start=(kc == 0), stop=(kc == 1))
                    o_sb = pO.tile([128, 512], BF16)
                    if alt and m % 2 == 1:
                        # Act is idle during the final proj chain: bias-add
                        # there so the DVE chain is not the tail's critical path
                        nc.scalar.activation(
                            o_sb[:], ps[:], mybir.ActivationFunctionType.Identity,
                            bias=b_pr_sb[:, m:m + 1])
                    else:
                        nc.vector.tensor_scalar_add(o_sb[:], ps[:],
                                                    b_pr_sb[:, m:m + 1])
                    eng = nc.scalar if (alt and m % 2 == 1) else nc.sync
                    eng.dma_start(
                        bounce_n[qi][m * 128:(m + 1) * 128, :], o_sb[:])
                    if m == 7:
                        nc.gpsimd.collective_compute(
                            "ReduceScatter", mybir.AluOpType.add,
                            replica_groups=GROUPS,
                            ins=[bounce_n[qi][:]], outs=[rs_out_n[qi][:]])
                        if alt:
                            # end of kernel: sync/scalar queues are empty, so
                            # their post-collective waits can't block anything
                            nc.sync.dma_start(
                                out_rs[0:128, qi * 512:(qi + 1) * 512],
                                rs_out_n[qi][0:128, :])
                            nc.scalar.dma_start(
                                out_rs[128:256, qi * 512:(qi + 1) * 512],
                                rs_out_n[qi][128:256, :])
                        else:
                            nc.gpsimd.dma_start(
                                out_rs[:, qi * 512:(qi + 1) * 512],
                                rs_out_n[qi][:])
                return run
            out.append(mk(m))
        return out

    # ---- main pipeline ----------------------------------------------------
    add_a_fillers(0)
    fillers.append((107.0, bvp_run, 2))
    add_b_fillers(0)
    add_a_fillers(1)
    add_b_fillers(1)
    add_a_fillers(2)
    add_b_fillers(2)
    add_a_fillers(3)
    add_b_fillers(3)

    carry_d = []
    for qi in (0, 1, 2, 3):
        drain_fillers(upto_tag=3 * qi)       # A of heads 0/1 for chunks <= qi
        nch = 4 * (qi + 1)
        gi = 0
        tail_q = []
        y2 = {}
        for h in range(HL):
            if h == 2:
                drain_fillers(upto_tag=3 * qi + 1)
            p_tiles = []
            y_ps = psPV.tile([128, 4 * 65], F32)
            for j in range(nch):
                rel = j - 4 * qi
                off = 128 * rel if rel >= 0 else 0
                s_ps = psS.tile([128, 512], F32)
                nc.tensor.matmul(
                    s_ps[:, off:], lhsT=k_ap(h)[:, j * 128:(j + 1) * 128],
                    rhs=q_ap(h)[:, qi * 512 + off:(qi + 1) * 512],
                    start=True, stop=(rel < 0))
                rows = 512 - off
                state["pe"] += rows * PE_ROW
                if rel >= 0:
                    # accumulate the additive causal mask into the diagonal
                    # 128-col block, then close the accumulation group
                    nc.tensor.matmul(
                        s_ps[:, off:off + 128], lhsT=ident[:], rhs=maskneg[:],
                        start=False, stop=True, skip_group_check=True)
                    state["pe"] += 128 * PE_ROW
                state["act"] = max(state["act"], state["pe"] + EXP_LAT) \
                    + rows * ACT_ROW + ACT_FIX
                p_sb = pP.tile([128, 512], BF16)
                nc.scalar.activation(p_sb[:, off:], s_ps[:, off:], Exp,
                                     scale=EXP_SCALE)
                if h == 0 and j == 4 * qi:
                    drain_fillers(upto_tag=3 * qi + 2)   # V of chunk qi
                p_tiles.append(p_sb)
                if tail_q:
                    tail_q.pop(0)()
                emit_fillers()
                gi += 1
                if carry_d:
                    carry_d.pop(0)()
                    state["pe"] += 426.0
                if h == 3 and fillers and fillers[0][2] <= 3 * qi + 5:
                    cost, run, _t = fillers.pop(0)
                    run()
                    state["pe"] += cost
            # defer this head's PV burst + epilogue; each closure is one
            # complete PSUM accumulation group (kept consecutive) or the
            # divide/transpose chain, popped during the next head's j-loop
            def mk_pv(h, y_ps, p_tiles, s):
                def run():
                    for j in range(4 * qi + s + 1):
                        nc.tensor.matmul(
                            y_ps[:, 65 * s:65 * s + 65],
                            lhsT=p_tiles[j][:, 128 * s:128 * s + 128],
                            rhs=V_sb[j][:, 65 * h:65 * h + 65],
                            start=(j == 0), stop=(j == 4 * qi + s))
                        state["pe"] += 27.0
                return run

            def mk_ep(h, y_ps):
                def run():
                    rec = pR.tile([128, 4], F32)
                    nc.vector.reciprocal(
                        rec[:],
                        y_ps[:].rearrange("p (s c) -> p s c", s=4)[:, :, 64])
                    col = (h % 2) * 64
                    for s in range(4):
                        if h % 2 == 0:
                            y2[s] = pY.tile([128, 128], BF16, name="y2")
                        nc.vector.tensor_scalar_mul(
                            y2[s][:, col:col + 64], y_ps[:, 65 * s:65 * s + 64],
                            rec[:, s:s + 1])
                    if h % 2 == 1:
                        kcb = h // 2
                        for s in range(4):
                            eng = nc.sync if s % 2 == 0 else nc.scalar
                            eng.dma_start_transpose(
                                yT_sb[kcb][:, qi * 512 + 128 * s:
                                            qi * 512 + 128 * s + 128],
                                y2[s][:])
                return run

            for s in range(4):
                tail_q.append(mk_pv(h, y_ps, p_tiles, s))
            tail_q.append(mk_ep(h, y_ps))
        for run in tail_q:
            run()
        for run in carry_d:
            run()
        carry_d = d_closures(qi, alt=(qi == NT - 1))
    for run in carry_d:
        run()
    drain_fillers()


_NC_CACHE = None


def _get_nc():
    global _NC_CACHE
    if _NC_CACHE is None:
        _NC_CACHE = _build_bass()
    return _NC_CACHE


def kernel(x, w_qkv, b_qkv, w_proj, b_proj, **_kw):
    x = np.asarray(x, dtype=np.float32)
    w_qkv = np.asarray(w_qkv, dtype=np.float32)
    b_qkv = np.asarray(b_qkv, dtype=np.float32)
    w_proj = np.asarray(w_proj, dtype=np.float32)
    b_proj = np.asarray(b_proj, dtype=np.float32)

    nc = _get_nc()
    import ml_dtypes
    bf = ml_dtypes.bfloat16
    f8 = ml_dtypes.float8_e4m3

    def hilo(a):
        h = a.astype(f8)
        l = (a - h.astype(np.float32)).astype(f8)
        return np.ascontiguousarray(h), np.ascontiguousarray(l)

    in_maps = []
    for c in range(N_CORES):
        b = c // 4
        g = c % 4
        qs = slice(g * CL, (g + 1) * CL)
        ks = slice(C + g * CL, C + (g + 1) * CL)
        vs = slice(2 * C + g * CL, 2 * C + (g + 1) * CL)
        xh, xl = hilo(x[b].T)
        wqh, wql = hilo(
            np.concatenate([w_qkv[:, qs], w_qkv[:, ks]], axis=1) * SW_QK)
        wvh, wvl = hilo(w_qkv[:, vs] * SW_V)
        in_maps.append({
            "xT": xh, "xTl": xl,
            "w_qk": wqh, "w_qkl": wql,
            "b_qk": np.ascontiguousarray(
                np.concatenate([b_qkv[qs], b_qkv[ks]]) * SW_QK),
            "w_v": wvh, "w_vl": wvl,
            "b_v": np.ascontiguousarray(b_qkv[vs] * SW_V),
            "w_pr": np.ascontiguousarray(w_proj[g * CL:(g + 1) * CL, :].astype(bf)),
            "b_pr": b_proj if g == 0 else np.zeros_like(b_proj),
        })

    res = bass_utils.run_bass_kernel_spmd(nc, in_maps, core_ids=list(range(N_CORES)))

    out = np.empty((B, T, C), dtype=np.float32)
    for b in range(B):
        projT = np.concatenate(
            [np.asarray(res.results[4 * b + r]["out_rs"], dtype=np.float32)
             for r in range(4)], axis=0)
        out[b] = projT.T
    return out


if __name__ == "__main__":
    rng = np.random.RandomState(0)
    ins = {
        "x": rng.randn(B, T, C).astype(np.float32),
        "w_qkv": rng.randn(C, 3 * C).astype(np.float32) / 32,
        "b_qkv": rng.randn(3 * C).astype(np.float32) / 32,
        "w_proj": rng.randn(C, C).astype(np.float32) / 32,
        "b_proj": rng.randn(C).astype(np.float32) / 32,
    }
    y = kernel(**ins)
    print("kernel ran, out shape", y.shape)


# revision 40
# speedup vs baseline: 1.4604x; 1.4604x over previous
"""Causal self-attention (B=2, T=2048, C=1024, H=16) on 8 trn2 NeuronCores.

Sharding: data-parallel over batch (2 groups of 4 cores) x tensor-parallel over
heads (4 heads / core).  v10: QKV projections as fp8e4 DoubleRow matmuls with
hi/lo splitting (each operand = hi + lo fp8 residual, 3 cross products -> full
bf16-class precision at 0.75x the bf16 PE cost; weights pre-scaled x256/x128
against fp8 subnormals, rescale folded into the exp scale and the V ones
column), bf16 scores/P/V/proj, causal mask via a second PE matmul accumulating
-1e8 into diagonal score blocks, paired-head transpose epilogue with merged
yT copies, PE p-state warmup matmuls over the initial DMA window, prioritized
first-chunk DMAs, 3-buffer score PSUM (B-filler accumulator shares the psW
pool), per-chunk ReduceScatters with post-collective DMAs routed off the busy
queues, and Act-assisted final proj bias-adds.
"""

import sys

for _p in ("/opt/trn_rl_repo",):
    if _p not in sys.path:
        sys.path.append(_p)

import numpy as np
from contextlib import ExitStack

import concourse.bass as bass
import concourse.mybir as mybir
import concourse.tile as tile
from concourse import bass_utils

B, T, C, H = 2, 2048, 1024, 16
D = C // H              # 64
N_CORES = 8
GROUPS = [[0, 1, 2, 3], [4, 5, 6, 7]]
HL = 4                  # heads per core
CL = HL * D             # 256 local channels
KC = C // 128           # 8 contraction chunks of 128
NT = T // 512           # 4 token chunks of 512
TOKC = T // 128         # 16 token chunks of 128
F32 = mybir.dt.float32
F32R = mybir.dt.float32r
BF16 = mybir.dt.bfloat16
FP8 = mybir.dt.float8e4
DRMODE = mybir.MatmulPerfMode.DoubleRow
SW_QK = 256.0           # host scale on w_qk/b_qk (fp8 subnormal avoidance)
SW_V = 128.0            # host scale on w_v/b_v
EXP_SCALE = 0.125 / (SW_QK * SW_QK)   # 2^-19 exactly
MASK_NEG = -1.0e8


def _legalize_waits(nc):
    """This walrus build allows at most ONE sync-wait per instruction. Move
    extra waits onto same-engine NoOps inserted just before the instruction."""
    n_split = 0
    for f in nc.m.functions:
        for bb in f.blocks:
            out = []
            for inst in bb.instructions:
                si = inst.sync_info
                waits = list(si.on_wait) if si is not None and si.on_wait else []
                if len(waits) > 1:
                    for i, w in enumerate(waits[:-1]):
                        out.append(
                            mybir.InstNoOp(
                                name=f"wsplit_{inst.name}_{i}",
                                engine=inst.engine,
                                ins=[],
                                outs=[],
                                sync_info=mybir.SyncInfo(on_wait=[w], on_update=[]),
                            )
                        )
                        n_split += 1
                    si.on_wait = [waits[-1]]
                out.append(inst)
            bb.instructions = out
    return n_split


def _build_bass():
    nc = bass.Bass("TRN2", target_bir_lowering=False, debug=False,
                   num_devices=N_CORES)

    xT = nc.dram_tensor("xT", [C, T], FP8, kind="ExternalInput").ap()
    xTl = nc.dram_tensor("xTl", [C, T], FP8, kind="ExternalInput").ap()
    w_qk = nc.dram_tensor("w_qk", [C, 2 * CL], FP8, kind="ExternalInput").ap()
    w_qkl = nc.dram_tensor("w_qkl", [C, 2 * CL], FP8, kind="ExternalInput").ap()
    b_qk = nc.dram_tensor("b_qk", [2 * CL], F32, kind="ExternalInput").ap()
    w_v = nc.dram_tensor("w_v", [C, CL], FP8, kind="ExternalInput").ap()
    w_vl = nc.dram_tensor("w_vl", [C, CL], FP8, kind="ExternalInput").ap()
    b_v = nc.dram_tensor("b_v", [CL], F32R, kind="ExternalInput").ap()
    w_pr = nc.dram_tensor("w_pr", [CL, C], BF16, kind="ExternalInput").ap()
    b_pr = nc.dram_tensor("b_pr", [C], F32, kind="ExternalInput").ap()
    out_rs = nc.dram_tensor("out_rs", [C // 4, T], BF16, kind="ExternalOutput").ap()

    with tile.TileContext(nc) as tc:
        with ExitStack() as ctx:
            with nc.allow_low_precision(reason="fp8/bf16 attention; tolerance 2e-2"):
                _build_body(ctx, tc, nc, xT, xTl, w_qk, w_qkl, b_qk, w_v, w_vl,
                            b_v, w_pr, b_pr, out_rs)

    _legalize_waits(nc)
    return nc


def _build_body(ctx, tc, nc, xT, xTl, w_qk, w_qkl, b_qk, w_v, w_vl, b_v,
                w_pr, b_pr, out_rs):
    Exp = mybir.ActivationFunctionType.Exp

    persist = ctx.enter_context(tc.tile_pool(name="persist", bufs=1))
    dram = ctx.enter_context(tc.tile_pool(name="dram", bufs=1, space="DRAM"))

    # ---- pools (kept open for the whole kernel; no phase barriers) --------
    psW = ctx.enter_context(tc.tile_pool(name="psW", bufs=2, space="PSUM"))
    psS = ctx.enter_context(tc.tile_pool(name="psS", bufs=2, space="PSUM"))
    psPV = ctx.enter_context(tc.tile_pool(name="psPV", bufs=2, space="PSUM"))
    pP = ctx.enter_context(tc.tile_pool(name="pP", bufs=18))
    pR = ctx.enter_context(tc.tile_pool(name="pR", bufs=2))
    pY = ctx.enter_context(tc.tile_pool(name="pY", bufs=10))
    pO = ctx.enter_context(tc.tile_pool(name="pO", bufs=4))

    # ---- PE p-state warmup: dummy matmuls with no DMA deps ----------------
    dummy = persist.tile([128, 512], BF16, name="dummy")
    nc.gpsimd.memset(dummy[:], 0.5)
    for i in range(8):
        scr = psW.tile([128, 512], F32, name="a_ps")
        nc.tensor.matmul(scr[:], lhsT=dummy[:, 0:128], rhs=dummy[:],
                         start=True, stop=True)

    # ---- weight + activation loads, ordered so phase A starts earliest ----
    # fp8 hi/lo: each operand tile holds its hi fp8 part in k-planes 0..3 and
    # the lo residual in k-planes 4..7.  critical path: x(n=0) hi + w_qk hi
    # gate the first A DoubleRow products (hi x hi).
    w_qk_m = w_qk.rearrange("(g k p) c -> g p k c", g=2, k=4)
    w_qkl_m = w_qkl.rearrange("(g k p) c -> g p k c", g=2, k=4)
    xT_m = xT.rearrange("(g k p) (n c) -> n g p k c", g=2, k=4, n=NT)
    xTl_m = xTl.rearrange("(g k p) (n c) -> n g p k c", g=2, k=4, n=NT)
    xT_t = {}

    def load_x(n):
        for g in range(2):
            t = persist.tile([128, 8 * 512], FP8, name=f"xT_m{g}_{n}")
            k8 = t[:].rearrange("p (k c) -> p k c", k=8)
            eng = nc.scalar if (g == 1 and n == 0) else nc.sync
            eng.dma_start(k8[:, 0:4, :], xT_m[n, g])
            xT_t[g, n] = t
        for g in range(2):
            k8 = xT_t[g, n][:].rearrange("p (k c) -> p k c", k=8)
            eng = nc.scalar if (g == 1 and n == 0) else nc.sync
            eng.dma_start(k8[:, 4:8, :], xTl_m[n, g])

    def x_pair(hl, p, n, o, width):
        """[128, 2, width] fp8 DoubleRow operand: kc pair p, hi (0) / lo (1)."""
        t = xT_t[p // 2, n]
        k8 = t[:].rearrange("p (k c) -> p k c", k=8)
        base = 4 * hl + 2 * (p % 2)
        return k8[:, base:base + 2, o:o + width]

    class _PairSlices:
        def __init__(self, tiles, width):
            self.tiles, self.width = tiles, width

        def __getitem__(self, key):
            hl, p, o = key
            t = self.tiles[p // 2]
            k8 = t[:].rearrange("p (k c) -> p k c", k=8)
            base = 4 * hl + 2 * (p % 2)
            return k8[:, base:base + 2, o:o + self.width]

    # critical order: hi halves of x(n=0) + w_qk first, split across the two
    # DMA queues; lo halves right behind (only the 5th+ matmul needs them)
    w_qk_t = []
    t0 = persist.tile([128, 8 * 2 * CL], FP8, name="w_qk_m0")
    nc.sync.dma_start(t0[:].rearrange("p (k c) -> p k c", k=8)[:, 0:4, :],
                      w_qk_m[0])
    w_qk_t.append(t0)
    t1 = persist.tile([128, 8 * 2 * CL], FP8, name="w_qk_m1")
    load_x(0)   # hi g=0 on sync, hi g=1 on scalar, then lo halves
    nc.scalar.dma_start(t1[:].rearrange("p (k c) -> p k c", k=8)[:, 0:4, :],
                        w_qk_m[1])
    w_qk_t.append(t1)
    nc.sync.dma_start(t0[:].rearrange("p (k c) -> p k c", k=8)[:, 4:8, :],
                      w_qkl_m[0])
    nc.scalar.dma_start(t1[:].rearrange("p (k c) -> p k c", k=8)[:, 4:8, :],
                        w_qkl_m[1])
    w_qk_sb = _PairSlices(w_qk_t, 128)
    b_qk_sb = persist.tile([128, 4], F32, name="b_qk_sb")
    nc.sync.dma_start(b_qk_sb[:], b_qk.rearrange("(m p) -> p m", p=128))
    b_v_row = persist.tile([1, CL], F32R, name="b_v_row")
    nc.sync.dma_start(b_v_row[:], b_v.rearrange("(a c) -> a c", a=1))
    w_v_m = w_v.rearrange("(g k p) c -> g p k c", g=2, k=4)
    w_vl_m = w_vl.rearrange("(g k p) c -> g p k c", g=2, k=4)
    w_v_t = []
    for g in range(2):
        t = persist.tile([128, 8 * CL], FP8, name=f"w_v_m{g}")
        k8 = t[:].rearrange("p (k c) -> p k c", k=8)
        nc.sync.dma_start(k8[:, 0:4, :], w_v_m[g])
        nc.sync.dma_start(k8[:, 4:8, :], w_vl_m[g])
        w_v_t.append(t)
    w_v_sb = _PairSlices(w_v_t, CL)
    load_x(1)
    w_prb = []
    for kc in range(2):
        t = persist.tile([128, C], BF16, name=f"w_prb_{kc}")
        nc.sync.dma_start(t[:], w_pr[kc * 128:(kc + 1) * 128, :])
        w_prb.append(t)
    load_x(2)
    load_x(3)
    b_pr_sb = persist.tile([128, 8], F32, name="b_pr_sb")
    nc.sync.dma_start(b_pr_sb[:], b_pr.rearrange("(m p) -> p m", p=128))
    ones_f32 = persist.tile([1, 128], F32, name="ones_f32")
    nc.gpsimd.memset(ones_f32[:], 1.0)
    ones_row = persist.tile([1, 128], F32R, name="ones_row")
    nc.vector.tensor_copy(ones_row[:], ones_f32[:])

    # additive causal mask for the diagonal 128x128 block:
    # maskneg[k,q] = 0 if q>=k else -1e8 (applied via accumulating matmul)
    maskneg = persist.tile([128, 128], BF16, name="maskneg")
    nc.gpsimd.memset(maskneg[:], 0.0)
    nc.gpsimd.affine_select(
        out=maskneg[:], in_=maskneg[:], compare_op=mybir.AluOpType.is_ge,
        fill=MASK_NEG, base=0, pattern=[[1, 128]], channel_multiplier=-1)

    # paired copy for masking both head-planes of a score pair tile
    maskneg2 = persist.tile([128, 2, 128], BF16, name="maskneg2")
    nc.gpsimd.memset(maskneg2[:], 0.0)
    nc.gpsimd.affine_select(
        out=maskneg2[:], in_=maskneg2[:], compare_op=mybir.AluOpType.is_ge,
        fill=MASK_NEG, base=0, pattern=[[0, 2], [1, 128]], channel_multiplier=-1)

    # bf16 identity for PE mask matmuls, f32 identity for f32 transposes
    ident = persist.tile([128, 128], BF16, name="ident")
    nc.gpsimd.memset(ident[:], 1.0)
    nc.gpsimd.affine_select(
        out=ident[:], in_=ident[:], compare_op=mybir.AluOpType.is_ge, fill=0.0,
        base=0, pattern=[[1, 128]], channel_multiplier=-1)
    nc.gpsimd.affine_select(
        out=ident[:], in_=ident[:], compare_op=mybir.AluOpType.is_ge, fill=0.0,
        base=0, pattern=[[-1, 128]], channel_multiplier=1)
    ident32 = persist.tile([128, 128], F32, name="ident32")
    nc.gpsimd.memset(ident32[:], 1.0)
    nc.gpsimd.affine_select(
        out=ident32[:], in_=ident32[:], compare_op=mybir.AluOpType.is_ge,
        fill=0.0, base=0, pattern=[[1, 128]], channel_multiplier=-1)
    nc.gpsimd.affine_select(
        out=ident32[:], in_=ident32[:], compare_op=mybir.AluOpType.is_ge,
        fill=0.0, base=0, pattern=[[-1, 128]], channel_multiplier=1)

    # ---- persistent intermediates ----------------------------------------
    # QK_sb[m]: m=0,1 -> Q channels (heads 0,1 | 2,3), m=2,3 -> K channels
    QK_sb = [persist.tile([128, T], BF16, name=f"QK_{m}") for m in range(4)]
    # V in [tok, ch] bf16 layout, 65 cols/head: col h*65+64 is the ones column
    # denominator carries the same x128 scale as the V data columns)
    V_sb = [persist.tile([128, HL * 65], BF16, name=f"V_{t}") for t in range(TOKC)]
    # y^T in bf16, rows = local channels (kc 0: heads 0,1; kc 1: heads 2,3)
    yT_sb = [persist.tile([128, T], BF16, name=f"yT_{i}") for i in range(2)]

    # denominator column carries the same xSW_V scale as the V data columns
    for t in range(TOKC):
        vt = V_sb[t][:].rearrange("p (h c) -> p h c", h=HL)
        nc.gpsimd.memset(vt[:, :, 64:65], SW_V)

    # broadcast b_v across partitions via a rank-1 matmul (emitted as a
    # filler, tag 2, so its b_v_row DMA wait never stalls the early PE stream)
    b_v_bc = persist.tile([128, CL], F32, name="b_v_bc")

    def bvp_run():
        bvp = psW.tile([128, 512], F32, name="a_ps")
        nc.tensor.matmul(bvp[:, 0:CL], lhsT=ones_row[:], rhs=b_v_row[:],
                         start=True, stop=True)
        nc.vector.tensor_copy(b_v_bc[:], bvp[:, 0:CL])

    bounce_n = [dram.tile([C, 512], BF16, name=f"bounce_{n}") for n in range(NT)]
    rs_out_n = [dram.tile([C // 4, 512], BF16, name=f"rs_out_{n}") for n in range(NT)]

    def q_ap(h):
        return QK_sb[h // 2][(h % 2) * 64:(h % 2) * 64 + 64, :]

    def k_ap(h):
        return QK_sb[2 + h // 2][(h % 2) * 64:(h % 2) * 64 + 64, :]

    # ---- static scheduler: PE is the master stream; exps (Act) must never
    # starve.  Fillers are single matmuls injected whenever the PE virtual
    # clock is ahead of the Act virtual clock.
    PE_ROW = 1.0 / 2.4          # ns per output row at full p-state
    ACT_ROW = 1.0 / 1.2
    ACT_FIX = 185.0
    EXP_LAT = 250.0             # sem hop from score-done to exp start

    state = {"pe": 0.0, "act": 0.0}
    fillers = []                # list of (cost_ns, closure, tag)

    def emit_fillers(margin=800.0):
        if state["act"] == 0.0:
            return
        while fillers and state["pe"] < state["act"] + margin:
            cost, run, _tag = fillers.pop(0)
            run()
            state["pe"] += cost

    def drain_fillers(upto_tag=None):
        while fillers and (upto_tag is None or fillers[0][2] is not None
                           and fillers[0][2] <= upto_tag):
            cost, run = fillers.pop(0)[:2]
            run()
            state["pe"] += cost

    # A(m,n): 12 fp8 DoubleRow matmuls (kc pairs x products hh, hl, lh)
    # accumulating into one psW tile, then DVE bias-add into bf16 QK.
    # m order 0,2 (heads 0/1 Q+K) then 1,3 so attention can start earliest.
    A_PRODS = [(0, 0, p) for p in range(4)] + \
              [(0, 1, p) for p in range(4)] + \
              [(1, 0, p) for p in range(4)]
    NA = len(A_PRODS)

    def add_a_fillers(n):
        holder = {}
        for gi, grp in enumerate(((0, 2), (1, 3))):
            for m in grp:
                def mk(m, i):
                    w_hl, x_hl, p = A_PRODS[i]

                    def run():
                        if i == 0:
                            holder[m] = psW.tile([128, 512], F32, name="a_ps")
                        nc.tensor.matmul(
                            holder[m][:], lhsT=w_qk_sb[w_hl, p, m * 128],
                            rhs=x_pair(x_hl, p, n, 0, 512),
                            start=(i == 0), stop=(i == NA - 1),
                            perf_mode=DRMODE)
                        if i == NA - 1:
                            nc.vector.tensor_scalar_add(
                                QK_sb[m][:, n * 512:(n + 1) * 512], holder.pop(m)[:],
                                b_qk_sb[:, m:m + 1])
                    return run
                for i in range(NA):
                    fillers.append((107.0, mk(m, i), 3 * n + gi))

    # B(t,n): 12 DoubleRow matmuls into psW, then DVE bias-add into V bf16.
    B_PRODS = [(0, 0, p) for p in range(4)] + \
              [(1, 0, p) for p in range(4)] + \
              [(0, 1, p) for p in range(4)]

    def add_b_fillers(n):
        holder = {}
        for t in range(4):
            def mk(t, i):
                x_hl, w_hl, p = B_PRODS[i]

                def run():
                    if i == 0:
                        holder[t] = psW.tile([128, 512], F32, name="a_ps")
                    nc.tensor.matmul(
                        holder[t][:, 0:CL],
                        lhsT=x_pair(x_hl, p, n, t * 128, 128),
                        rhs=w_v_sb[w_hl, p, 0],
                        start=(i == 0), stop=(i == NA - 1),
                        perf_mode=DRMODE)
                    if i == NA - 1:
                        vt = V_sb[n * 4 + t][:].rearrange("p (h c) -> p h c", h=HL)
                        nc.vector.tensor_add(
                            vt[:, :, 0:64],
                            holder.pop(t)[:, 0:CL].rearrange(
                                "p (h c) -> p h c", h=HL),
                            b_v_bc[:].rearrange("p (h c) -> p h c", h=HL))
                return run
            for i in range(NA):
                fillers.append((53.0, mk(t, i), 3 * n + 2))

    # D(m,qi): 2 kc-matmuls + DVE bias-add + DMA; last m also issues the RS.
    # The post-RS output DMA goes on the gpsimd (SWDGE) queue: its wait on the
    # collective must not head-of-line-block the bounce DMA queues.
    def d_closures(qi, alt=False):
        out = []
        for m in range(8):
            def mk(m):
                def run():
                    ps = psW.tile([128, 512], F32, name="a_ps")
                    for kc in range(2):
                        nc.tensor.matmul(
                            ps[:], lhsT=w_prb[kc][:, m * 128:(m + 1) * 128],
                            rhs=yT_sb[kc][:, qi * 512:(qi + 1) * 512],
                            start=(kc == 0), stop=(kc == 1))
                    o_sb = pO.tile([128, 512], BF16)
                    if alt and m % 2 == 1:
                        # Act is idle during the final proj chain: bias-add
                        # there so the DVE chain is not the tail's critical path
                        nc.scalar.activation(
                            o_sb[:], ps[:], mybir.ActivationFunctionType.Identity,
                            bias=b_pr_sb[:, m:m + 1])
                    else:
                        nc.vector.tensor_scalar_add(o_sb[:], ps[:],
                                                    b_pr_sb[:, m:m + 1])
                    eng = nc.scalar if (alt and m % 2 == 1) else nc.sync
                    eng.dma_start(
                        bounce_n[qi][m * 128:(m + 1) * 128, :], o_sb[:])
                    if m == 7:
                        nc.gpsimd.collective_compute(
                            "ReduceScatter", mybir.AluOpType.add,
                            replica_groups=GROUPS,
                            ins=[bounce_n[qi][:]], outs=[rs_out_n[qi][:]])
                        if alt:
                            # end of kernel: sync/scalar queues are empty, so
                            # their post-collective waits can't block anything
                            nc.sync.dma_start(
                                out_rs[0:128, qi * 512:(qi + 1) * 512],
                                rs_out_n[qi][0:128, :])
                            nc.scalar.dma_start(
                                out_rs[128:256, qi * 512:(qi + 1) * 512],
                                rs_out_n[qi][128:256, :])
                        else:
                            nc.gpsimd.dma_start(
                                out_rs[:, qi * 512:(qi + 1) * 512],
                                rs_out_n[qi][:])
                return run
            out.append(mk(m))
        return out

    # ---- main pipeline ----------------------------------------------------
    add_a_fillers(0)
    fillers.append((107.0, bvp_run, 2))
    add_b_fillers(0)
    add_a_fillers(1)
    add_b_fillers(1)
    add_a_fillers(2)
    add_b_fillers(2)
    add_a_fillers(3)
    add_b_fillers(3)

    carry_d = []
    for qi in (0, 1, 2, 3):
        drain_fillers(upto_tag=3 * qi)       # A of heads 0/1 for chunks <= qi
        nch = 4 * (qi + 1)
        gi = 0
        tail_q = []
        y2 = {}
        for hp in range(2):
            h0, h1 = 2 * hp, 2 * hp + 1
            if hp == 1:
                drain_fillers(upto_tag=3 * qi + 1)
            p_tiles = []
            y_ps_d = {h0: psPV.tile([128, 4 * 65], F32, name="y_ps"),
                      h1: psPV.tile([128, 4 * 65], F32, name="y_ps")}
            for j in range(nch):
                rel = j - 4 * qi
                off = 128 * rel if rel >= 0 else 0
                s_ps = psS.tile([128, 1024], F32)
                for hl, h in ((0, h0), (1, h1)):
                    nc.tensor.matmul(
                        s_ps[:, 512 * hl + off:512 * (hl + 1)],
                        lhsT=k_ap(h)[:, j * 128:(j + 1) * 128],
                        rhs=q_ap(h)[:, qi * 512 + off:(qi + 1) * 512],
                        start=True, stop=(rel < 0))
                    rows = 512 - off
                    state["pe"] += rows * PE_ROW
                    if rel >= 0:
                        nc.tensor.matmul(
                            s_ps[:, 512 * hl + off:512 * hl + off + 128],
                            lhsT=ident[:], rhs=maskneg[:],
                            start=False, stop=True, skip_group_check=True)
                        state["pe"] += 128 * PE_ROW
                state["act"] = max(state["act"], state["pe"] + EXP_LAT) \
                    + 2 * rows * ACT_ROW + ACT_FIX
                p_sb = pP.tile([128, 1024], BF16)
                nc.scalar.activation(
                    p_sb[:].rearrange("p (hl c) -> p hl c", hl=2)[:, :, off:],
                    s_ps[:].rearrange("p (hl c) -> p hl c", hl=2)[:, :, off:],
                    Exp, scale=EXP_SCALE)
                if hp == 0 and j == 4 * qi:
                    drain_fillers(upto_tag=3 * qi + 2)   # V of chunk qi
                p_tiles.append(p_sb)
                for _ in range(2):
                    if tail_q:
                        tail_q.pop(0)()
                emit_fillers()
                gi += 1
                if carry_d:
                    carry_d.pop(0)()
                    state["pe"] += 426.0
                if hp == 1 and fillers and fillers[0][2] <= 3 * qi + 5:
                    cost, run, _t = fillers.pop(0)
                    run()
                    state["pe"] += cost
            # defer this head's PV burst + epilogue; each closure is one
            # complete PSUM accumulation group (kept consecutive) or the
            # divide/transpose chain, popped during the next head's j-loop
            def mk_pv(h, y_ps, p_tiles, s):
                def run():
                    for j in range(4 * qi + s + 1):
                        nc.tensor.matmul(
                            y_ps[:, 65 * s:65 * s + 65],
                            lhsT=p_tiles[j][:, 512 * (h % 2) + 128 * s:
                                            512 * (h % 2) + 128 * s + 128],
                            rhs=V_sb[j][:, 65 * h:65 * h + 65],
                            start=(j == 0), stop=(j == 4 * qi + s))
                        state["pe"] += 27.0
                return run

            def mk_ep(h, y_ps):
                def run():
                    rec = pR.tile([128, 4], F32)
                    nc.vector.reciprocal(
                        rec[:],
                        y_ps[:].rearrange("p (s c) -> p s c", s=4)[:, :, 64])
                    col = (h % 2) * 64
                    for s in range(4):
                        if h % 2 == 0:
                            y2[s] = pY.tile([128, 128], F32, name="y2")
                        nc.vector.tensor_scalar_mul(
                            y2[s][:, col:col + 64], y_ps[:, 65 * s:65 * s + 64],
                            rec[:, s:s + 1])
                    if h % 2 == 1:
                        kcb = h // 2
                        t_ps = psW.tile([128, 512], F32, name="a_ps")
                        for s in range(4):
                            nc.tensor.transpose(
                                t_ps[:, 128 * s:128 * s + 128], y2[s][:],
                                ident32[:])
                            state["pe"] += 107.0
                        nc.vector.tensor_copy(
                            yT_sb[kcb][:, qi * 512:(qi + 1) * 512], t_ps[:])
                return run

            for h in (h0, h1):
                for s in range(4):
                    tail_q.append(mk_pv(h, y_ps_d[h], p_tiles, s))
                tail_q.append(mk_ep(h, y_ps_d[h]))
        for run in tail_q:
            run()
        for run in carry_d:
            run()
        carry_d = d_closures(qi, alt=(qi == NT - 1))
    for run in carry_d:
        run()
    drain_fillers()


_NC_CACHE = None


def _get_nc():
    global _NC_CACHE
    if _NC_CACHE is None:
        _NC_CACHE = _build_bass()
    return _NC_CACHE


def kernel(x, w_qkv, b_qkv, w_proj, b_proj, **_kw):
    x = np.asarray(x, dtype=np.float32)
    w_qkv = np.asarray(w_qkv, dtype=np.float32)
    b_qkv = np.asarray(b_qkv, dtype=np.float32)
    w_proj = np.asarray(w_proj, dtype=np.float32)
    b_proj = np.asarray(b_proj, dtype=np.float32)

    nc = _get_nc()
    import ml_dtypes
    bf = ml_dtypes.bfloat16
    f8 = ml_dtypes.float8_e4m3

    def hilo(a):
        h = a.astype(f8)
        l = (a - h.astype(np.float32)).astype(f8)
        return np.ascontiguousarray(h), np.ascontiguousarray(l)

    in_maps = []
    for c in range(N_CORES):
        b = c // 4
        g = c % 4
        qs = slice(g * CL, (g + 1) * CL)
        ks = slice(C + g * CL, C + (g + 1) * CL)
        vs = slice(2 * C + g * CL, 2 * C + (g + 1) * CL)
        xh, xl = hilo(x[b].T)
        wqh, wql = hilo(
            np.concatenate([w_qkv[:, qs], w_qkv[:, ks]], axis=1) * SW_QK)
        wvh, wvl = hilo(w_qkv[:, vs] * SW_V)
        in_maps.append({
            "xT": xh, "xTl": xl,
            "w_qk": wqh, "w_qkl": wql,
            "b_qk": np.ascontiguousarray(
                np.concatenate([b_qkv[qs], b_qkv[ks]]) * SW_QK),
            "w_v": wvh, "w_vl": wvl,
            "b_v": np.ascontiguousarray(b_qkv[vs] * SW_V),
            "w_pr": np.ascontiguousarray(w_proj[g * CL:(g + 1) * CL, :].astype(bf)),
            "b_pr": b_proj if g == 0 else np.zeros_like(b_proj),
        })

    res = bass_utils.run_bass_kernel_spmd(nc, in_maps, core_ids=list(range(N_CORES)))

    out = np.empty((B, T, C), dtype=np.float32)
    for b in range(B):
        projT = np.concatenate(
            [np.asarray(res.results[4 * b + r]["out_rs"], dtype=np.float32)
             for r in range(4)], axis=0)
        out[b] = projT.T
    return out


if __name__ == "__main__":
    rng = np.random.RandomState(0)
    ins = {
        "x": rng.randn(B, T, C).astype(np.float32),
        "w_qkv": rng.randn(C, 3 * C).astype(np.float32) / 32,
        "b_qkv": rng.randn(3 * C).astype(np.float32) / 32,
        "w_proj": rng.randn(C, C).astype(np.float32) / 32,
        "b_proj": rng.randn(C).astype(np.float32) / 32,
    }
    y = kernel(**ins)
    print("kernel ran, out shape", y.shape)
